# revision 6
# baseline (speedup 1.0000x reference)
"""Trainium2 kernel: block-circulant FFT linear layer (bf16, pair-packed pivots,
ACT triple / DVE single PSUM rings).

Over v3 (u64 DVE elements are ISA-illegal, so pairs stay the pivot unit):
  - pipeline order fixed: T2(r) is emitted before fwd(r+1), so inv(r)
    never waits behind fwd(r+1)'s DVE work
  - s2 layout [b, k, s, w2] (wp = 2b+s): T1 out is only 2-strided and
    the mid stage reads 8B contiguous runs (balance of v2/v3 extremes)
  - evac split ACT 32 / DVE 16; T1(r+1) emitted after inv(r) so the
    DVE queue serves inv's evacuations when the PE needs them

kernel(x, W): x [4096, 4096] f32, W [64, 64, 64] f32 -> [4096, 4096] f32.
"""
import numpy as np
import ml_dtypes
import concourse.bass as bass
import concourse.bacc as bacc
import concourse.mybir as mybir
import concourse.tile as tile
from concourse.bass_utils import run_bass_kernel_spmd

N_CORES = 8
B, IN, OUT, BS = 4096, 4096, 4096, 64
BC = B // N_CORES            # 512 batch rows per core
WP = BC // 2                 # 256 w-pairs per core
NA = 32
NK = 32
WC = 64                      # w-pairs per transpose/DMA chunk
NWQ = WP // WC               # 4 chunks
BF16 = mybir.dt.bfloat16
F32 = mybir.dt.float32


# ---------------- host-side constant matrices ----------------

def make_fmat():
    t = np.arange(BS)[:, None]
    c = np.arange(BS)[None, :]
    k = np.where(c <= 32, c, c - 32)
    ang = 2 * np.pi * k * t / BS
    F = np.where(c <= 32, np.cos(ang), np.sin(ang))
    bd = np.zeros((128, 128), np.float32)
    bd[:64, :64] = F
    bd[64:, 64:] = F
    return bd.astype(ml_dtypes.bfloat16)


def make_gmat():
    tau = np.arange(BS)[None, :]
    c = np.arange(BS)[:, None]
    k = np.where(c <= 32, c, c - 32)
    ang = 2 * np.pi * k * tau / BS
    base = np.where(c <= 32, np.cos(ang), np.sin(ang))
    scale = np.where((c % 32) == 0, 1.0 / BS, 2.0 / BS)
    G = base * scale
    bd = np.zeros((128, 128), np.float32)
    bd[:64, :64] = G
    bd[64:, 64:] = G
    return bd.astype(ml_dtypes.bfloat16)


def make_wmats(W):
    W = np.asarray(W, np.float32)
    s = np.arange(BS)
    k = np.arange(33)
    ang = 2 * np.pi * k[:, None] * s[None, :] / BS
    wr = np.einsum("ijs,ks->ijk", W, np.cos(ang))
    wi = np.einsum("ijs,ks->ijk", W, np.sin(ang))
    M = np.zeros((NK, 128, 128), np.float32)
    icol = np.empty(64, np.int64)
    for i in range(64):
        a, par = divmod(i, 2)
        icol[i] = 64 * par + a
    for kk in range(NK):
        if kk == 0:
            W32 = wr[:, :, 32]
        Wr, Wi = wr[:, :, kk], wi[:, :, kk]
        for par_j in range(2):
            jrows = np.arange(32) * 2 + par_j
            rre = 64 * par_j + np.arange(32)
            rim = rre + 32
            for i in range(64):
                cre = icol[i]
                cim = cre + 32
                if kk == 0:
                    M[0, rre, cre] = wr[i, jrows, 0]
                    M[0, rim, cim] = W32[i, jrows]
                else:
                    M[kk, rre, cre] = Wr[i, jrows]
                    M[kk, rim, cre] = -Wi[i, jrows]
                    M[kk, rre, cim] = Wi[i, jrows]
                    M[kk, rim, cim] = Wr[i, jrows]
    return np.ascontiguousarray(M.transpose(1, 0, 2)).astype(ml_dtypes.bfloat16)


def prep_x(x):
    """[B, 4096] f32 -> per-core [128, WP, NA, 2] bf16.

    partition p = par*64 + t (j = 2a+par); free = (w-pair, a, w-parity)."""
    xr = np.asarray(x, np.float32).reshape(N_CORES, WP, 2, NA, 2, 64)
    xp = xr.transpose(0, 4, 5, 1, 3, 2)      # [c, par, t, wp, a, w2]
    return np.ascontiguousarray(xp).reshape(
        N_CORES, 128, WP, NA, 2).astype(ml_dtypes.bfloat16)


def post_y(ys):
    """per-core [128, WP, NA, 2] bf16 -> [B, 4096] f32; p = par*64 + tau,
    i = 2a+par."""
    y = np.stack(ys).astype(np.float32)      # [c, 128, WP, NA, 2]
    y = y.reshape(N_CORES, 2, 64, WP, NA, 2)  # [c, par, tau, wp, a, w2]
    y = y.transpose(0, 3, 5, 4, 1, 2)        # [c, wp, w2, a, par, tau]
    return np.ascontiguousarray(y).reshape(B, OUT)


# ---------------- device kernel ----------------

def build_nc(reps=1):
    nc = bacc.Bacc("TRN2", target_bir_lowering=False, debug=False,
                   num_devices=N_CORES, dynamic_dma_scratch_size=8192)
    x_in = nc.dram_tensor("x", [128, WP, NA, 2], BF16, kind="ExternalInput")
    fmat = nc.dram_tensor("fmat", [128, 128], BF16, kind="ExternalInput")
    gmat = nc.dram_tensor("gmat", [128, 128], BF16, kind="ExternalInput")
    wmat = nc.dram_tensor("wmat", [128, NK, 128], BF16, kind="ExternalInput")
    y_out = nc.dram_tensor("y", [128, WP, NA, 2], BF16, kind="ExternalOutput")

    with tile.TileContext(nc) as tc:
        with (
            tc.tile_pool(name="consts", bufs=1) as cpool,
            tc.tile_pool(name="px", bufs=2) as px,
            tc.tile_pool(name="pt", bufs=1) as pt,
            tc.tile_pool(name="psa", bufs=2, space="PSUM") as psa,
            tc.tile_pool(name="psd", bufs=2, space="PSUM") as psd,
        ):
            f_sb = cpool.tile([128, 128], BF16)
            g_sb = cpool.tile([128, 128], BF16)
            w_sb = cpool.tile([128, NK, 128], BF16)
            nc.sync.dma_start(f_sb[:], fmat[:])
            nc.sync.dma_start(g_sb[:], gmat[:])
            nc.sync.dma_start(w_sb[:], wmat[:])

            def alloc_x(r):
                xh = []
                for h in range(2):
                    xt = px.tile([128, WP // 2, NA, 2], BF16, tag="x",
                                 name=f"xt{h}")
                    nc.sync.dma_start(
                        xt[:], x_in[:, h * (WP // 2):(h + 1) * (WP // 2)])
                    xh.append(xt)
                return xh

            def emit_fwd(xh):
                s_sb = pt.tile([128, WP, NA, 2], BF16, tag="s", name="s_sb")
                # s2: wp = 2*b + s; [b, k, s, w2] so T1-out is 2-strided in
                # k and the mid stage reads 8B-contiguous (s, w2) runs
                s2 = pt.tile([128, WP // 2, NK, 2, 2], BF16, tag="s2",
                             name="s2")
                for wq in range(NWQ):
                    h, hw = wq // 2, (wq % 2) * WC
                    ws = slice(wq * WC, (wq + 1) * WC)
                    for base, nj in ((0, 3), (12, 1), (16, 3), (28, 1)):
                        if nj == 3:
                            ps = psa.tile([128, 3, WC, 4, 2], F32, tag="ps3",
                                          name="psf")
                            for j in range(3):
                                a0 = base + j * 4
                                nc.tensor.matmul(
                                    ps[:, j], f_sb[:],
                                    xh[h][:, hw:hw + WC, a0:a0 + 4, :])
                            nc.scalar.copy(
                                s_sb[:, ws, base:base + 12, :].rearrange(
                                    "p w (j a) c -> p j w a c", j=3),
                                ps[:])
                        else:
                            ps = psd.tile([128, WC, 4, 2], F32, tag="ps1",
                                          name="psf")
                            nc.tensor.matmul(
                                ps[:], f_sb[:],
                                xh[h][:, hw:hw + WC, base:base + 4, :])
                            nc.vector.tensor_copy(
                                s_sb[:, ws, base:base + 4, :], ps[:])
                return s_sb, s2

            def emit_t1(s_sb, s2):
                for wq in range(NWQ):
                    ws = slice(wq * WC, (wq + 1) * WC)
                    bs = slice(wq * (WC // 2), (wq + 1) * (WC // 2))
                    nc.vector.transpose(
                        s2[:, bs].bitcast(F32).rearrange(
                            "p b k s x -> p b s k x"),
                        s_sb[:, ws].bitcast(F32))

            def emit_mid(s2):
                o_sb = pt.tile([128, WP, NK, 2], BF16, tag="o", name="o_sb")
                for g4 in range(NK // 4):
                    k0 = g4 * 4
                    ps = psa.tile([128, 3, WP // 2, 2, 2], F32, tag="ps3",
                                  name="psm")
                    for j in range(3):
                        nc.tensor.matmul(
                            ps[:, j], w_sb[:, k0 + j, :],
                            s2[:, :, k0 + j, :, :])
                    nc.scalar.copy(
                        o_sb[:, :, k0:k0 + 3, :].rearrange(
                            "p (b s) k c -> p k b s c", s=2),
                        ps[:])
                    ps1 = psd.tile([128, WP // 2, 2, 2], F32, tag="ps1",
                                   name="psm1")
                    nc.tensor.matmul(
                        ps1[:], w_sb[:, k0 + 3, :], s2[:, :, k0 + 3, :, :])
                    nc.vector.tensor_copy(
                        o_sb[:, :, k0 + 3, :].rearrange(
                            "p (b s) c -> p b s c", s=2),
                        ps1[:])
                return o_sb

            def emit_t2(o_sb):
                v_sb = pt.tile([128, WP, NA, 2], BF16, tag="v", name="v_sb")
                for wq in range(NWQ):
                    ws = slice(wq * WC, (wq + 1) * WC)
                    nc.vector.transpose(
                        v_sb[:, ws].bitcast(F32), o_sb[:, ws].bitcast(F32))
                return v_sb

            def emit_inv(v_sb, pending_out):
                y_sb = pt.tile([128, WP, NA, 2], BF16, tag="y", name="y_sb")
                for wq in range(NWQ):
                    ws = slice(wq * WC, (wq + 1) * WC)
                    groups = ((0, 3), (12, 1), (16, 3), (28, 1)) if wq < 3 \
                        else ((0, 1), (4, 1), (8, 1), (12, 1), (16, 3), (28, 1))
                    for base, nj in groups:
                        if nj == 3:
                            ps = psa.tile([128, 3, WC, 4, 2], F32, tag="ps3",
                                          name="psi")
                            for j in range(3):
                                a0 = base + j * 4
                                nc.tensor.matmul(
                                    ps[:, j], g_sb[:],
                                    v_sb[:, ws, a0:a0 + 4, :])
                            nc.scalar.copy(
                                y_sb[:, ws, base:base + 12, :].rearrange(
                                    "p w (j a) c -> p j w a c", j=3),
                                ps[:])
                        else:
                            ps = psd.tile([128, WC, 4, 2], F32, tag="ps1",
                                          name="psi")
                            nc.tensor.matmul(
                                ps[:], g_sb[:],
                                v_sb[:, ws, base:base + 4, :])
                            nc.vector.tensor_copy(
                                y_sb[:, ws, base:base + 4, :], ps[:])
                    pending_out.append((y_out[:, ws], y_sb[:, ws]))

            # software pipeline: PE order [mid(r), fwd(r+1), inv(r)];
            # DVE order [mid-casts(r), T2(r), fwd-casts+T1(r+1), inv-casts(r)]
            pending_out = []
            xh = alloc_x(0)
            s_sb, s2 = emit_fwd(xh)
            emit_t1(s_sb, s2)
            for r in range(reps):
                if r + 1 < reps:
                    xh = alloc_x(r + 1)
                o_sb = emit_mid(s2)
                v_sb = emit_t2(o_sb)
                if r + 1 < reps:
                    s_sb, s2 = emit_fwd(xh)
                emit_inv(v_sb, pending_out)
                if r + 1 < reps:
                    emit_t1(s_sb, s2)
                for dst, src in pending_out:
                    nc.sync.dma_start(dst, src)
                pending_out = []

    nc.compile()
    return nc


_NC_CACHE = {}


def _in_maps(x, W):
    fmat = make_fmat()
    gmat = make_gmat()
    wmat = make_wmats(W)
    xp = prep_x(x)
    return [
        {"x": xp[c], "fmat": fmat, "gmat": gmat, "wmat": wmat}
        for c in range(N_CORES)
    ]


def run(x, W, reps=1):
    if reps not in _NC_CACHE:
        _NC_CACHE[reps] = build_nc(reps)
    nc = _NC_CACHE[reps]
    res = run_bass_kernel_spmd(nc, _in_maps(x, W), list(range(N_CORES)))
    return post_y([res.results[c]["y"] for c in range(N_CORES)])


def kernel(x, W):
    if 1 not in _NC_CACHE:
        _NC_CACHE[1] = build_nc(reps=1)
    res = run_bass_kernel_spmd(nc=_NC_CACHE[1], in_maps=_in_maps(x, W),
                               core_ids=list(range(N_CORES)))
    return post_y([res.results[c]["y"] for c in range(N_CORES)])


# revision 7
# speedup vs baseline: 1.0752x; 1.0752x over previous
"""Trainium2 kernel: block-circulant FFT linear layer (bf16, pair-packed pivots,
ACT triple / DVE single PSUM rings).

Over v3 (u64 DVE elements are ISA-illegal, so pairs stay the pivot unit):
  - pipeline order fixed: T2(r) is emitted before fwd(r+1), so inv(r)
    never waits behind fwd(r+1)'s DVE work
  - s2 layout [b, k, s, w2] (wp = 2b+s): T1 out is only 2-strided and
    the mid stage reads 8B contiguous runs (balance of v2/v3 extremes)
  - evac split ACT 32 / DVE 16; T1(r+1) emitted after inv(r) so the
    DVE queue serves inv's evacuations when the PE needs them

kernel(x, W): x [4096, 4096] f32, W [64, 64, 64] f32 -> [4096, 4096] f32.
"""
import numpy as np
import ml_dtypes
import concourse.bass as bass
import concourse.bacc as bacc
import concourse.mybir as mybir
import concourse.tile as tile
from concourse.bass_utils import run_bass_kernel_spmd

N_CORES = 8
B, IN, OUT, BS = 4096, 4096, 4096, 64
BC = B // N_CORES            # 512 batch rows per core
WP = BC // 2                 # 256 w-pairs per core
NA = 32
NK = 32
WC = 64                      # w-pairs per transpose/DMA chunk
NWQ = WP // WC               # 4 chunks
BF16 = mybir.dt.bfloat16
F32 = mybir.dt.float32


# ---------------- host-side constant matrices ----------------

def make_fmat():
    t = np.arange(BS)[:, None]
    c = np.arange(BS)[None, :]
    k = np.where(c <= 32, c, c - 32)
    ang = 2 * np.pi * k * t / BS
    F = np.where(c <= 32, np.cos(ang), np.sin(ang))
    bd = np.zeros((128, 128), np.float32)
    bd[:64, :64] = F
    bd[64:, 64:] = F
    return bd.astype(ml_dtypes.bfloat16)


def make_gmat():
    tau = np.arange(BS)[None, :]
    c = np.arange(BS)[:, None]
    k = np.where(c <= 32, c, c - 32)
    ang = 2 * np.pi * k * tau / BS
    base = np.where(c <= 32, np.cos(ang), np.sin(ang))
    scale = np.where((c % 32) == 0, 1.0 / BS, 2.0 / BS)
    G = base * scale
    bd = np.zeros((128, 128), np.float32)
    bd[:64, :64] = G
    bd[64:, 64:] = G
    return bd.astype(ml_dtypes.bfloat16)


def make_wmats(W):
    W = np.asarray(W, np.float32)
    s = np.arange(BS)
    k = np.arange(33)
    ang = 2 * np.pi * k[:, None] * s[None, :] / BS
    wr = np.einsum("ijs,ks->ijk", W, np.cos(ang))
    wi = np.einsum("ijs,ks->ijk", W, np.sin(ang))
    M = np.zeros((NK, 128, 128), np.float32)
    icol = np.empty(64, np.int64)
    for i in range(64):
        a, par = divmod(i, 2)
        icol[i] = 64 * par + a
    for kk in range(NK):
        if kk == 0:
            W32 = wr[:, :, 32]
        Wr, Wi = wr[:, :, kk], wi[:, :, kk]
        for par_j in range(2):
            jrows = np.arange(32) * 2 + par_j
            rre = 64 * par_j + np.arange(32)
            rim = rre + 32
            for i in range(64):
                cre = icol[i]
                cim = cre + 32
                if kk == 0:
                    M[0, rre, cre] = wr[i, jrows, 0]
                    M[0, rim, cim] = W32[i, jrows]
                else:
                    M[kk, rre, cre] = Wr[i, jrows]
                    M[kk, rim, cre] = -Wi[i, jrows]
                    M[kk, rre, cim] = Wi[i, jrows]
                    M[kk, rim, cim] = Wr[i, jrows]
    return np.ascontiguousarray(M.transpose(1, 0, 2)).astype(ml_dtypes.bfloat16)


def prep_x(x):
    """[B, 4096] f32 -> per-core [128, WP, NA, 2] bf16.

    partition p = par*64 + t (j = 2a+par); free = (w-pair, a, w-parity)."""
    xr = np.asarray(x, np.float32).reshape(N_CORES, WP, 2, NA, 2, 64)
    xp = xr.transpose(0, 4, 5, 1, 3, 2)      # [c, par, t, wp, a, w2]
    return np.ascontiguousarray(xp).reshape(
        N_CORES, 128, WP, NA, 2).astype(ml_dtypes.bfloat16)


def post_y(ys):
    """per-core [128, WP, NA, 2] bf16 -> [B, 4096] f32; p = par*64 + tau,
    i = 2a+par."""
    y = np.stack(ys).astype(np.float32)      # [c, 128, WP, NA, 2]
    y = y.reshape(N_CORES, 2, 64, WP, NA, 2)  # [c, par, tau, wp, a, w2]
    y = y.transpose(0, 3, 5, 4, 1, 2)        # [c, wp, w2, a, par, tau]
    return np.ascontiguousarray(y).reshape(B, OUT)


# ---------------- device kernel ----------------

def build_nc(reps=1):
    nc = bacc.Bacc("TRN2", target_bir_lowering=False, debug=False,
                   num_devices=N_CORES, dynamic_dma_scratch_size=8192)
    x_in = nc.dram_tensor("x", [128, WP, NA, 2], BF16, kind="ExternalInput")
    fmat = nc.dram_tensor("fmat", [128, 128], BF16, kind="ExternalInput")
    gmat = nc.dram_tensor("gmat", [128, 128], BF16, kind="ExternalInput")
    wmat = nc.dram_tensor("wmat", [128, NK, 128], BF16, kind="ExternalInput")
    y_out = nc.dram_tensor("y", [128, WP, NA, 2], BF16, kind="ExternalOutput")

    with tile.TileContext(nc) as tc:
        with (
            tc.tile_pool(name="consts", bufs=1) as cpool,
            tc.tile_pool(name="px", bufs=2) as px,
            tc.tile_pool(name="pt", bufs=1) as pt,
            tc.tile_pool(name="psa", bufs=2, space="PSUM") as psa,
            tc.tile_pool(name="psd", bufs=2, space="PSUM") as psd,
        ):
            f_sb = cpool.tile([128, 128], BF16)
            g_sb = cpool.tile([128, 128], BF16)
            w_sb = cpool.tile([128, NK, 128], BF16)
            nc.sync.dma_start(f_sb[:], fmat[:])
            nc.sync.dma_start(g_sb[:], gmat[:])
            nc.sync.dma_start(w_sb[:], wmat[:])

            def alloc_x(r):
                xh = []
                for h in range(2):
                    xt = px.tile([128, WP // 2, NA, 2], BF16, tag="x",
                                 name=f"xt{h}")
                    nc.sync.dma_start(
                        xt[:], x_in[:, h * (WP // 2):(h + 1) * (WP // 2)])
                    xh.append(xt)
                return xh

            def emit_fwd(xh):
                s_sb = pt.tile([128, WP, NA, 2], BF16, tag="s", name="s_sb")
                # s2: wp = 2*b + s; [b, k, s, w2] so T1-out is 2-strided in
                # k and the mid stage reads 8B-contiguous (s, w2) runs
                s2 = pt.tile([128, WP // 2, NK, 2, 2], BF16, tag="s2",
                             name="s2")
                for wq in range(NWQ):
                    h, hw = wq // 2, (wq % 2) * WC
                    ws = slice(wq * WC, (wq + 1) * WC)
                    for base, nj in ((0, 3), (12, 1), (16, 3), (28, 1)):
                        if nj == 3:
                            ps = psa.tile([128, 3, WC, 4, 2], F32, tag="ps3",
                                          name="psf")
                            for j in range(3):
                                a0 = base + j * 4
                                nc.tensor.matmul(
                                    ps[:, j], f_sb[:],
                                    xh[h][:, hw:hw + WC, a0:a0 + 4, :])
                            nc.scalar.copy(
                                s_sb[:, ws, base:base + 12, :].rearrange(
                                    "p w (j a) c -> p j w a c", j=3),
                                ps[:])
                        else:
                            ps = psd.tile([128, WC, 4, 2], F32, tag="ps1",
                                          name="psf")
                            nc.tensor.matmul(
                                ps[:], f_sb[:],
                                xh[h][:, hw:hw + WC, base:base + 4, :])
                            nc.vector.tensor_copy(
                                s_sb[:, ws, base:base + 4, :], ps[:])
                return s_sb, s2

            def emit_t1(s_sb, s2):
                for wq in range(NWQ):
                    ws = slice(wq * WC, (wq + 1) * WC)
                    bs = slice(wq * (WC // 2), (wq + 1) * (WC // 2))
                    nc.vector.transpose(
                        s2[:, bs].bitcast(F32).rearrange(
                            "p b k s x -> p b s k x"),
                        s_sb[:, ws].bitcast(F32))

            def emit_mid(s2):
                o_sb = pt.tile([128, WP, NK, 2], BF16, tag="o", name="o_sb")
                for g4 in range(NK // 4):
                    k0 = g4 * 4
                    ps = psa.tile([128, 3, WP // 2, 2, 2], F32, tag="ps3",
                                  name="psm")
                    for j in range(3):
                        nc.tensor.matmul(
                            ps[:, j], w_sb[:, k0 + j, :],
                            s2[:, :, k0 + j, :, :])
                    nc.scalar.copy(
                        o_sb[:, :, k0:k0 + 3, :].rearrange(
                            "p (b s) k c -> p k b s c", s=2),
                        ps[:])
                    ps1 = psd.tile([128, WP // 2, 2, 2], F32, tag="ps1",
                                   name="psm1")
                    nc.tensor.matmul(
                        ps1[:], w_sb[:, k0 + 3, :], s2[:, :, k0 + 3, :, :])
                    nc.vector.tensor_copy(
                        o_sb[:, :, k0 + 3, :].rearrange(
                            "p (b s) c -> p b s c", s=2),
                        ps1[:])
                return o_sb

            def emit_t2(o_sb):
                v_sb = pt.tile([128, WP, NA, 2], BF16, tag="v", name="v_sb")
                for wq in range(NWQ):
                    ws = slice(wq * WC, (wq + 1) * WC)
                    nc.vector.transpose(
                        v_sb[:, ws].bitcast(F32), o_sb[:, ws].bitcast(F32))
                return v_sb

            def emit_inv(v_sb, pending_out):
                y_sb = pt.tile([128, WP, NA, 2], BF16, tag="y", name="y_sb")
                for wq in range(NWQ):
                    ws = slice(wq * WC, (wq + 1) * WC)
                    for base, nj in ((0, 3), (12, 1), (16, 3), (28, 1)):
                        if nj == 3:
                            ps = psa.tile([128, 3, WC, 4, 2], F32, tag="ps3",
                                          name="psi")
                            for j in range(3):
                                a0 = base + j * 4
                                nc.tensor.matmul(
                                    ps[:, j], g_sb[:],
                                    v_sb[:, ws, a0:a0 + 4, :])
                            nc.scalar.copy(
                                y_sb[:, ws, base:base + 12, :].rearrange(
                                    "p w (j a) c -> p j w a c", j=3),
                                ps[:])
                        else:
                            ps = psd.tile([128, WC, 4, 2], F32, tag="ps1",
                                          name="psi")
                            nc.tensor.matmul(
                                ps[:], g_sb[:],
                                v_sb[:, ws, base:base + 4, :])
                            nc.vector.tensor_copy(
                                y_sb[:, ws, base:base + 4, :], ps[:])
                    pending_out.append((y_out[:, ws], y_sb[:, ws]))

            # software pipeline: PE order [mid(r), fwd(r+1), inv(r)];
            # DVE order [mid-casts(r), T2(r), fwd-casts+T1(r+1), inv-casts(r)]
            pending_out = []
            xh = alloc_x(0)
            s_sb, s2 = emit_fwd(xh)
            emit_t1(s_sb, s2)
            for r in range(reps):
                if r + 1 < reps:
                    xh = alloc_x(r + 1)
                o_sb = emit_mid(s2)
                v_sb = emit_t2(o_sb)
                if r + 1 < reps:
                    s_sb, s2 = emit_fwd(xh)
                emit_inv(v_sb, pending_out)
                if r + 1 < reps:
                    emit_t1(s_sb, s2)
                for dst, src in pending_out:
                    nc.sync.dma_start(dst, src)
                pending_out = []

    nc.compile()
    return nc


_NC_CACHE = {}


def _in_maps(x, W):
    fmat = make_fmat()
    gmat = make_gmat()
    wmat = make_wmats(W)
    xp = prep_x(x)
    return [
        {"x": xp[c], "fmat": fmat, "gmat": gmat, "wmat": wmat}
        for c in range(N_CORES)
    ]


def run(x, W, reps=1):
    if reps not in _NC_CACHE:
        _NC_CACHE[reps] = build_nc(reps)
    nc = _NC_CACHE[reps]
    res = run_bass_kernel_spmd(nc, _in_maps(x, W), list(range(N_CORES)))
    return post_y([res.results[c]["y"] for c in range(N_CORES)])


def kernel(x, W):
    if 1 not in _NC_CACHE:
        _NC_CACHE[1] = build_nc(reps=1)
    res = run_bass_kernel_spmd(nc=_NC_CACHE[1], in_maps=_in_maps(x, W),
                               core_ids=list(range(N_CORES)))
    return post_y([res.results[c]["y"] for c in range(N_CORES)])
